# revision 21
# baseline (speedup 1.0000x reference)
"""Trainium2 Bass kernel for BaseCausalWanSelfAttention (local+sink sparse attention
with interleaved rotary), SPMD across 8 NeuronCores.

Sharding: the 24 (batch, head) pairs are split 3-per-core across 8 cores; each
core runs full local+sink attention for its pairs independently (no collectives).

Design (v3):
 - all-bf16 inputs (host casts); rotary on DVE using host-supplied row-swapped
   copies of qT/kT (no PE swap matmuls, 2x DVE mode).
 - scores [k, q] layout; QK in bf16; per query-block (512 q) the k-tiles are
   processed in PAIRS sharing one 2-bank PSUM tile; one Exp ACTIVATE per pair
   over the union chunk-range (2D AP) amortizes ACT instruction overhead.
 - masks (causal diag / window edges / out-of-range chunks) applied as ONE
   combined additive -30000 matmul per masked k-tile into the score PSUM
   (ident stationary, precomputed pattern moving), so exp yields exact zeros.
 - PV and denominator matmuls in fp8e4m3 with DoubleRow perf mode (2 k-tiles
   per matmul at 0.5 cyc/col); probs written by ACT directly in fp8 with an
   exp bias of -1.5 to keep values in fp8 range (cancels in normalization).
 - query block 0 (s < 512) uses a bf16 PV/den path instead: early tokens have
   few attended keys, softmax is peaked there, and fp8 V quantization would
   pass straight through to the output.
 - output written transposed [d, S] per unit; host transposes back.
"""
import sys

sys.path.insert(0, "/opt/trn_rl_repo")

import ml_dtypes
import numpy as np

import concourse.bacc as bacc
import concourse.mybir as mybir
import concourse.tile as tile

dt = mybir.dt
BF16 = ml_dtypes.bfloat16
FP8 = ml_dtypes.float8_e4m3

B, S, H, D = 2, 3072, 12, 128
LOCAL_WINDOW = 1560
SINK = 128
N_CORES = 8
PER_CORE = (B * H) // N_CORES  # 3
QB = 512
NQC = QB // 128  # 4
NKT = S // 128  # 24
NQB = S // QB  # 6
MAXD = 13  # max k-tile delta with any valid element (w=1560)
SCALE = 1.0 / float(np.sqrt(D))
MASK_NEG = -30000.0
EXP_BIAS = -1.5
PROBS_FP8 = True
BF16_QB0 = True  # query block 0 uses bf16 probs/V (fp8 noise too visible there)

PROB_DT = dt.float8e4 if PROBS_FP8 else dt.bfloat16
DRMODE = mybir.MatmulPerfMode.DoubleRow

# Schraudolph fast-exp constants (exp(x) ~= bitcast_f32(int32(A*x + B)));
# folded with the score scale and exp bias. Used to offload a slice of the
# exp work from the saturated scalar engine to the vector engine.
SCHR_A = float((2 ** 23) / np.log(2) * SCALE)
SCHR_B = float(127 * (2 ** 23) - 486411 + (2 ** 23) / np.log(2) * EXP_BIAS)


def offload_exp(qb, g, npairs, u0, u1):
    return False


def kj_list(qb):
    lo = max(1, NQC * qb - MAXD)
    hi = min(NKT - 1, NQC * qb + NQC - 1)
    return [0] + list(range(lo, hi + 1))


def tile_range(qb, kj):
    """Valid chunk range [t0, t1) of k-tile kj within query block qb."""
    if kj == 0:
        return 0, NQC
    t0 = max(0, kj - NQC * qb)
    t1 = min(NQC, kj + MAXD - NQC * qb + 1)
    return t0, t1


def chunk_code(qb, kj, t):
    qi = NQC * qb + t
    if kj == 0:
        return "D" if qi == 0 else "."
    d = qi - kj
    if d < 0 or d > MAXD:
        return "F"
    if d == 0:
        return "D"
    if d == 12:
        return "W12"
    if d == 13:
        return "W13"
    return "."


def pair_info(qb):
    """[(kjA, kjB, u0, u1, [(j, patkey), ...]), ...] for query block qb."""
    kjs = kj_list(qb)
    assert len(kjs) % 2 == 0
    pairs = []
    for i in range(0, len(kjs), 2):
        a, b = kjs[i], kjs[i + 1]
        ra, rb = tile_range(qb, a), tile_range(qb, b)
        u0, u1 = min(ra[0], rb[0]), max(ra[1], rb[1])
        ms = []
        for j, kj in ((0, a), (1, b)):
            pat = tuple(chunk_code(qb, kj, t) for t in range(u0, u1))
            if any(c != "." for c in pat):
                ms.append((j, pat))
        pairs.append((a, b, u0, u1, ms))
    assert pairs[0][2] == 0 and pairs[0][3] == NQC
    return pairs


def all_patterns():
    pats = {}
    for qb in range(NQB):
        for _, _, _, _, ms in pair_info(qb):
            for _, pat in ms:
                if pat not in pats:
                    pats[pat] = f"cm{len(pats)}"
    return pats


PATTERNS = all_patterns()


def build_nc(s=S, per_core=PER_CORE):
    nc = bacc.Bacc("TRN2", target_bir_lowering=False, debug=False)

    qk4 = nc.declare_dram_parameter("qk4", [per_core, 128, 4 * s], dt.bfloat16, isOutput=False)
    vp = nc.declare_dram_parameter("vp", [per_core, 128, s], PROB_DT, isOutput=False)
    vhead = nc.declare_dram_parameter("vhead", [per_core, 128, QB], dt.bfloat16, isOutput=False)
    tables = nc.declare_dram_parameter("tables", [128, 2 * s], dt.bfloat16, isOutput=False)
    mbD = nc.declare_dram_parameter("mbD", [128, 128], dt.bfloat16, isOutput=False)
    cmask_d = {
        pat: nc.declare_dram_parameter(nm, [128, 128 * len(pat)], dt.bfloat16, isOutput=False)
        for pat, nm in PATTERNS.items()
    }
    ident = nc.declare_dram_parameter("ident", [128, 128], dt.bfloat16, isOutput=False)
    ebias = nc.declare_dram_parameter("ebias", [128, 1], dt.float32, isOutput=False)
    ones2 = nc.declare_dram_parameter("ones2", [128, 256], PROB_DT, isOutput=False)
    ones2b = nc.declare_dram_parameter("ones2b", [128, 256], dt.bfloat16, isOutput=False)
    outD = nc.declare_dram_parameter("outD", [per_core, 128, s], dt.bfloat16, isOutput=True)

    with tile.TileContext(nc) as tc:
        with (
            tc.tile_pool(name="const", bufs=1) as cpool,
            tc.tile_pool(name="unit", bufs=2) as upool,
            tc.tile_pool(name="probs", bufs=6) as ppool,
            tc.tile_pool(name="probsA", bufs=2) as papool,
            tc.tile_pool(name="tail", bufs=2) as tpool,
            tc.tile_pool(name="schr", bufs=2) as spool,
            tc.tile_pool(name="ps_sc", bufs=3, space="PSUM") as ps_sc,
            tc.tile_pool(name="ps_out", bufs=1, space="PSUM") as ps_out,
            tc.tile_pool(name="ps_den", bufs=1, space="PSUM") as ps_den,
        ):
            tbl_sb = cpool.tile([128, 2 * s], dt.bfloat16, tag="tbl")
            cexp_sb = tbl_sb[:, 0:s]
            ssig_sb = tbl_sb[:, s : 2 * s]
            ident_sb = cpool.tile([128, 128], dt.bfloat16, tag="ident")
            ebias_sb = cpool.tile([128, 1], dt.float32, tag="ebias")
            ones2_sb = cpool.tile([128, 256], PROB_DT, tag="ones2")
            ones2b_sb = cpool.tile([128, 256], dt.bfloat16, tag="ones2b")
            mbD_sb = cpool.tile([128, 128], dt.bfloat16, tag="mbD")
            cmask_sb = {
                pat: cpool.tile([128, 128 * len(pat)], dt.bfloat16, tag=nm, name=nm)
                for pat, nm in PATTERNS.items()
            }

            def load_small_consts(t):
                nc.scalar.dma_start(out=ebias_sb[:], in_=ebias[:])
                nc.scalar.dma_start(out=mbD_sb[:], in_=mbD[:])
                nc.scalar.dma_start(out=t["vh"][:], in_=vhead[0][:])
                nc.scalar.dma_start(out=ones2b_sb[:], in_=ones2b[:])
                nc.scalar.dma_start(out=ones2_sb[:], in_=ones2[:])
                nc.scalar.dma_start(out=ident_sb[:], in_=ident[:])

            def load_cmasks():
                first_use = {}
                for qb in range(1, NQB):
                    for _, _, _, _, ms in pair_info(qb):
                        for _, pat in ms:
                            first_use.setdefault(pat, qb)
                for pat in sorted(cmask_sb, key=lambda p: first_use.get(p, 99)):
                    nc.gpsimd.dma_start(out=cmask_sb[pat][:], in_=cmask_d[pat][:])

            def alloc_unit(u):
                qk = upool.tile([128, 4 * s], dt.bfloat16, tag="qk", name=f"qk{u}")
                t = {
                    nm: upool.tile(
                        [128, QB] if nm == "vh" else [128, s],
                        PROB_DT if nm == "v" else dt.bfloat16,
                        tag=nm,
                        name=f"{nm}{u}",
                    )
                    for nm in ("rq", "rk", "v", "vh")
                }
                t["qk"] = qk
                for i, nm in enumerate(("qraw", "qsw", "kraw", "ksw")):
                    t[nm] = qk[:, i * s : (i + 1) * s]
                return t

            def load_unit0(t):
                """Unit 0: interleave const and input chunks so rotary/attention
                start as early as possible."""
                qk4v = t["qk"][:].rearrange("p (f c) -> p f c", f=4)
                qk4s = qk4[0].rearrange("p (f c) -> p f c", f=4)
                tblv = tbl_sb[:].rearrange("p (f c) -> p f c", f=2)
                tbls = tables[:].rearrange("p (f c) -> p f c", f=2)

                def qk_chunk(eng, c):
                    sl = slice(c * 512, (c + 1) * 512)
                    eng.dma_start(out=qk4v[:, :, sl], in_=qk4s[:, :, sl])

                # qk chunks in order on the sync ring; tables and small
                # consts on the scalar ring; cmasks and v on the gpsimd ring.
                qk_chunk(nc.sync, 0)
                nc.scalar.dma_start(out=tblv[:, :, 0:512], in_=tbls[:, :, 0:512])
                nc.scalar.dma_start(out=tblv[:, :, 512:1536], in_=tbls[:, :, 512:1536])
                qk_chunk(nc.sync, 1)
                load_small_consts(t)
                load_cmasks()
                qk_chunk(nc.sync, 2)
                nc.scalar.dma_start(out=tblv[:, :, 1536:s], in_=tbls[:, :, 1536:s])
                qk_chunk(nc.sync, 3)
                nc.gpsimd.dma_start(out=t["v"][:, 0:1536], in_=vp[0][:, 0:1536])
                qk_chunk(nc.sync, 4)
                qk_chunk(nc.sync, 5)
                nc.gpsimd.dma_start(out=t["v"][:, 1536:s], in_=vp[0][:, 1536:s])

            def load_unit(u, t):
                half = 2 * s
                nc.sync.dma_start(out=t["qk"][:, 0:half], in_=qk4[u][:, 0:half])
                nc.sync.dma_start(out=t["qk"][:, half : 4 * s], in_=qk4[u][:, half : 4 * s])
                nc.gpsimd.dma_start(out=t["v"][:], in_=vp[u][:])
                nc.gpsimd.dma_start(out=t["vh"][:], in_=vhead[u][:])

            def rot_ops(t):
                ops = []
                for c in range(NQB):
                    sl = slice(c * 512, (c + 1) * 512)
                    for raw, sw, r in (
                        (t["qraw"], t["qsw"], t["rq"]),
                        (t["kraw"], t["ksw"], t["rk"]),
                    ):
                        ops.append(lambda r=r, raw=raw, sl=sl: nc.vector.tensor_mul(
                            r[:, sl], raw[:, sl], cexp_sb[:, sl]))
                        ops.append(lambda sw=sw, sl=sl: nc.vector.tensor_mul(
                            sw[:, sl], sw[:, sl], ssig_sb[:, sl]))
                        ops.append(lambda r=r, sw=sw, sl=sl: nc.vector.tensor_add(
                            r[:, sl], r[:, sl], sw[:, sl]))
                return ops

            def rot_chunk(t, c):
                """Rotary for cols [c*512,(c+1)*512) of both q and k."""
                sl = slice(c * 512, (c + 1) * 512)
                for raw, sw, r in (
                    (t["qraw"], t["qsw"], t["rq"]),
                    (t["kraw"], t["ksw"], t["rk"]),
                ):
                    nc.vector.tensor_mul(r[:, sl], raw[:, sl], cexp_sb[:, sl])
                    nc.vector.tensor_mul(sw[:, sl], sw[:, sl], ssig_sb[:, sl])
                    nc.vector.tensor_add(r[:, sl], r[:, sl], sw[:, sl])

            state = {"pv": [], "pv_late": []}

            def flush_pv(n=1):
                for _ in range(min(n, len(state["pv"]))):
                    state["pv"].pop(0)()

            def attention(u, t, qb, side):
                pairs = pair_info(qb)
                npairs = len(pairs)
                bf_path = BF16_QB0 and qb == 0 and PROBS_FP8
                pdt = dt.bfloat16 if bf_path else PROB_DT
                pool = papool if bf_path else ppool
                v3 = t["v"].rearrange("p (n d) -> p n d", d=128)
                vh3 = t["vh"].rearrange("p (n d) -> p n d", d=128)
                qbctx = {}

                def get_acc():
                    if "outT" not in qbctx:
                        qbctx["outT"] = ps_out.tile(
                            [128, QB], dt.float32, tag="outT", name=f"outT{u}_{qb}"
                        )
                        qbctx["den"] = ps_den.tile(
                            [128, QB], dt.float32, tag="den", name=f"den{u}_{qb}"
                        )
                    return qbctx["outT"], qbctx["den"]

                for g, (kjA, kjB, u0, u1, ms) in enumerate(pairs):
                    offl = offload_exp(qb, g, npairs, u0, u1)
                    sc = ps_sc.tile(
                        [128, 2 * QB], dt.float32, tag="sc", name=f"sc{u}_{qb}_{g}"
                    )
                    masked = {0: False, 1: False}
                    if not bf_path:
                        for j, _ in ms:
                            masked[0] |= j in (0, 2)
                            masked[1] |= j in (1, 2)
                    for j, kj in ((0, kjA), (1, kjB)):
                        csl = slice(qb * QB + u0 * 128, qb * QB + u1 * 128)
                        osl = slice(j * QB + u0 * 128, j * QB + u1 * 128)
                        nc.tensor.matmul(
                            sc[:, osl], t["rk"][:, kj * 128 : (kj + 1) * 128],
                            t["rq"][:, csl], start=True, stop=not masked[j],
                        )
                    if not bf_path:
                        for j, pat in ms:
                            msl = slice(j * QB + u0 * 128, j * QB + u1 * 128)
                            nc.tensor.matmul(
                                sc[:, msl], ident_sb[:], cmask_sb[pat][:],
                                start=False, stop=True, skip_group_check=True,
                            )
                    probs = pool.tile(
                        [128, 2 * QB], pdt, tag="probs", name=f"pr{u}_{qb}_{g}"
                    )
                    if u0 == 0 and u1 == NQC:
                        sc3, pr3 = sc[:], probs[:]
                    else:
                        sc3 = sc[:].rearrange("p (j c) -> p j c", j=2)[:, :, u0 * 128 : u1 * 128]
                        pr3 = probs[:].rearrange("p (j c) -> p j c", j=2)[:, :, u0 * 128 : u1 * 128]
                    if offl:
                        ti = spool.tile(
                            [128, 2 * QB], dt.int32, tag="schr", name=f"ti{u}_{qb}_{g}"
                        )
                        nc.vector.tensor_scalar(
                            ti[:], sc3, SCHR_A, SCHR_B,
                            mybir.AluOpType.mult, mybir.AluOpType.add,
                        )
                        nc.vector.tensor_copy(pr3, ti[:].bitcast(dt.float32))
                    else:
                        nc.scalar.activation(
                            pr3, sc3, mybir.ActivationFunctionType.Exp,
                            scale=SCALE, bias=ebias_sb[:],
                        )
                    if bf_path:
                        for j, pat in ms:
                            for ti_, code in enumerate(pat):
                                tt = u0 + ti_
                                psl = slice(j * QB + tt * 128, j * QB + (tt + 1) * 128)
                                if code == "F":
                                    nc.vector.memset(probs[:, psl], 0.0)
                                elif code == "D":
                                    nc.vector.tensor_mul(
                                        probs[:, psl], probs[:, psl], mbD_sb[:]
                                    )
                                else:
                                    assert code == ".", code

                    def pv_emit(
                        g=g, kjA=kjA, kjB=kjB, u0=u0, u1=u1,
                        probs=probs, last=None, bf_path=bf_path,
                    ):
                        outT, den = get_acc()
                        last = not state["pv"] and not state["pv_late"]
                        rhs = probs[:].rearrange("p (j c) -> p j c", j=2)[
                            :, :, u0 * 128 : u1 * 128
                        ]
                        osl = slice(u0 * 128, u1 * 128)
                        if not bf_path:
                            dstep = kjB - kjA
                            vpair = v3[:, kjA : kjB + 1 : dstep, :]
                            o3 = ones2_sb[:].rearrange("p (j d) -> p j d", j=2)
                            nc.tensor.matmul(
                                outT[:, osl], vpair, rhs,
                                start=(g == 0), stop=last, perf_mode=DRMODE,
                            )
                            nc.tensor.matmul(
                                den[:, osl], o3, rhs,
                                start=(g == 0), stop=last, perf_mode=DRMODE,
                            )
                        else:
                            o3 = ones2b_sb[:].rearrange("p (j d) -> p j d", j=2)
                            for j, kj in ((0, kjA), (1, kjB)):
                                nc.tensor.matmul(
                                    outT[:, osl], vh3[:, kj, :], rhs[:, j],
                                    start=(g == 0 and j == 0), stop=(last and j == 1),
                                )
                                nc.tensor.matmul(
                                    den[:, osl], o3[:, j], rhs[:, j],
                                    start=(g == 0 and j == 0), stop=(last and j == 1),
                                )

                    if offl:
                        state["pv_late"].append(pv_emit)
                    else:
                        state["pv"].append(pv_emit)
                    if len(state["pv"]) > 2:
                        flush_pv()
                    if side:
                        side.pop(0)()
                flush_pv(10**9)
                while state["pv_late"]:
                    state["pv_late"].pop(0)()

                outT, den = get_acc()
                rden = tpool.tile([128, QB], dt.float32, tag="rden")
                nc.vector.reciprocal_approx_fast(rden[:], den[:])
                outN = tpool.tile([128, QB], dt.bfloat16, tag="outN")
                nc.vector.tensor_mul(outN[:], outT[:], rden[:])
                nc.gpsimd.dma_start(
                    out=outD[u][:, qb * QB : (qb + 1) * QB], in_=outN[:]
                )

            warm_sb = cpool.tile([128, 128], dt.bfloat16, tag="warm")
            nc.vector.memset(warm_sb[:], 0.0)
            warm_ps = ps_den.tile([128, QB], dt.float32, tag="den", name="warmps")
            for _ in range(12):
                nc.tensor.matmul(
                    warm_ps[:, 0:128], warm_sb[:], warm_sb[:], start=True, stop=True
                )

            cur = alloc_unit(0)
            load_unit0(cur)
            nxt = None
            side = []
            for u in range(per_core):
                for qb in range(NQB):
                    if u == 0:
                        rot_chunk(cur, qb)
                    attention(u, cur, qb, side)
                    if u == 0 and qb == 0 and per_core > 1:
                        nxt = alloc_unit(1)
                        load_unit(1, nxt)
                        side.extend(rot_ops(nxt))
                if u + 1 < per_core:
                    while side:
                        side.pop(0)()
                    cur = nxt
                    if u + 2 < per_core:
                        nxt = alloc_unit(u + 2)
                        load_unit(u + 2, nxt)
                        side.extend(rot_ops(nxt))
                    else:
                        nxt = None

    nc.compile()
    return nc


def host_prep(q, k, v, cos, sin, s=S):
    b, _, h, d = q.shape

    cexp = np.empty((128, s), dtype=np.float32)
    ssig = np.empty((128, s), dtype=np.float32)
    cexp[0::2, :] = cos.T
    cexp[1::2, :] = cos.T
    ssig[0::2, :] = -sin.T
    ssig[1::2, :] = sin.T

    ident = np.eye(128, dtype=np.float32)
    ones2 = np.ones((128, 256), dtype=np.float32)

    p = np.arange(128)[:, None]
    c = np.arange(128)[None, :]
    base = {
        ".": np.zeros((128, 128), dtype=np.float32),
        "D": np.where(c >= p, 0.0, MASK_NEG).astype(np.float32),
        "W12": np.where((c - p) < 24, 0.0, MASK_NEG).astype(np.float32),
        "W13": np.where((c - p) < -104, 0.0, MASK_NEG).astype(np.float32),
        "F": np.full((128, 128), MASK_NEG, dtype=np.float32),
    }
    cmasks = {
        nm: np.hstack([base[cc] for cc in pat]) for pat, nm in PATTERNS.items()
    }

    perm = np.arange(128) ^ 1
    units = [(bi, hi) for bi in range(b) for hi in range(h)]
    per = len(units) // N_CORES
    prob_np = FP8 if PROBS_FP8 else BF16
    in_maps = []
    for core in range(N_CORES):
        us = units[core * per : (core + 1) * per]
        qTc = np.stack([np.ascontiguousarray(q[bi, :, hi, :].T) for bi, hi in us])
        kTc = np.stack([np.ascontiguousarray(k[bi, :, hi, :].T) for bi, hi in us])
        # v rearranged to [128, n*128] with vr[p, n*128+d] = v[n*128+p, d]
        vc = np.stack(
            [
                np.ascontiguousarray(
                    v[bi, :, hi, :]
                    .reshape(s // 128, 128, 128)
                    .transpose(1, 0, 2)
                    .reshape(128, s)
                )
                for bi, hi in us
            ]
        )
        m = {
            "qk4": np.concatenate(
                [qTc, qTc[:, perm, :], kTc, kTc[:, perm, :]], axis=2
            ).astype(BF16),
            "vp": vc.astype(prob_np),
            "vhead": vc[:, :, 0:QB].astype(BF16),
            "tables": np.concatenate([cexp, ssig], axis=1).astype(BF16),
            "ebias": np.full((128, 1), EXP_BIAS, dtype=np.float32),
            "mbD": np.where(c >= p, 1.0, 0.0).astype(BF16),
            "ident": ident.astype(BF16),
            "ones2": ones2.astype(prob_np),
            "ones2b": ones2.astype(BF16),
        }
        for nm, msk in cmasks.items():
            m[nm] = msk.astype(BF16)
        in_maps.append(m)
    return in_maps, units


_NC_CACHE = {}


def kernel(q, k, v, cos, sin):
    from concourse.bass_utils import run_bass_kernel_spmd

    q = np.asarray(q, dtype=np.float32)
    k = np.asarray(k, dtype=np.float32)
    v = np.asarray(v, dtype=np.float32)
    cos = np.asarray(cos, dtype=np.float32)
    sin = np.asarray(sin, dtype=np.float32)

    if "nc" not in _NC_CACHE:
        _NC_CACHE["nc"] = build_nc()
    nc = _NC_CACHE["nc"]

    in_maps, units = host_prep(q, k, v, cos, sin)
    res = run_bass_kernel_spmd(nc, in_maps, core_ids=list(range(N_CORES)))

    b, s, h, d = q.shape
    full = np.empty((b, s, h, d), dtype=np.float32)
    per = len(units) // N_CORES
    for core in range(N_CORES):
        o = res.results[core]["outD"]  # [per, 128, s] bf16, transposed layout
        for i, (bi, hi) in enumerate(units[core * per : (core + 1) * per]):
            full[bi, :, hi, :] = o[i].T.astype(np.float32)
    return full
